# revision 31
# baseline (speedup 1.0000x reference)
"""Cross-attention Trainium2 kernel (8 NeuronCores, SPMD).

Problem: B=4, S=3072, SKV=1036, D_EMBED=1024, D_CROSS=768, H=8, d_head=128.

Sharding (per spec hint): data-parallel over batch x tensor-parallel over
heads. Core c -> (batch b = c//2, head half hh = c%2). Each core projects
only its 4 heads' Q/K/V dims (halves K/V projection work), runs attention
for its 4 heads over the full 3072 query rows, and applies the row-split
output projection (contraction over its own 512 embed dims) producing a
PARTIAL [3072, 1024] output. The all-reduce of the two partials per batch
happens in gather() on host (cores cannot talk to each other here).

Per-core device program (feature-on-partition, token-on-free):
  inputs arrive pre-cast to bf16 on host (halves HBM ingest, keeps DMAs on
  the hardware-descriptor path instead of gpsimd software casting).
  qT[d,s]  = WqT.T @ xT    (bf16; chunk 0 ei-outer so PE starts after
                            1/8th of Wq+x has landed)
  kT[d,t]  = WkT.T @ yT
  v[t,d]   = yT.T @ WvT
  attention, sc-outer / head-inner, 3-stage software pipeline:
    scores[t,s] = kT_h.T @ qT_h_slice       (psum)
    p = exp(scores / sqrt(128))             (ACT -> bf16 expT)
    Z row sums: bf16 DVE tree (7 adds) -> 2 ones-matmuls (psum zf)
    attnT_h[:, sc] = (sum_t v_h[t].T @ pT[t]) * 1/zf  (DVE reads psums)
  out[s,e] partial = attn[s, own] @ WoT[own] (+ cstB on the hh=0 core);
    emitted per finished s-chunk so the 12.6MB output streams during
    attention instead of draining at the end.
"""

import sys

sys.path.insert(0, "/opt/trn_rl_repo")

import math

import numpy as np
import ml_dtypes

import concourse.bass as bass
import concourse.mybir as mybir
import concourse.tile as tile
from concourse import bacc
from concourse.bass import ts, ds
from concourse.bass_utils import run_bass_kernel_spmd

N_CORES = 8
B, S, SKV = 4, 3072, 1036
DE, DC, H, DH = 1024, 768, 8, 128
HC = H // 2            # 4 heads per core
DEO = HC * DH          # 512 own embed dims
NT_FULL = SKV // 128   # 8 full t-tiles
T_REM = SKV - NT_FULL * 128  # 12
NT = NT_FULL + 1       # 9 t-tiles
NE = DE // 128         # 8 contraction chunks (embed)
NDI = DEO // 128       # 4 own-dim chunks
NCC = DC // 128        # 6 contraction chunks (cross)
NCH = S // 512         # 6 x-chunks of 512 (ap 512 hides LDWEIGHTS)
NSC = S // 512         # 6 s-chunks of 512
NS = S // 128          # 24 s-tiles
INV_SQRT_DH = 1.0 / math.sqrt(DH)

F32 = mybir.dt.float32
F32R = mybir.dt.float32r
BF16 = mybir.dt.bfloat16


def _t_width(ti):
    return 128 if ti < NT_FULL else T_REM


def build_bass():
    nc = bacc.Bacc("TRN2", target_bir_lowering=False, debug=False)

    xT_d = nc.dram_tensor("xT", [DE, S], BF16, kind="ExternalInput").ap()
    yT_d = nc.dram_tensor("yT", [DC, SKV], BF16, kind="ExternalInput").ap()
    wqT_d = nc.dram_tensor("WqT", [DE, DEO], BF16, kind="ExternalInput").ap()
    wkT_d = nc.dram_tensor("WkT", [DC, DEO], BF16, kind="ExternalInput").ap()
    wvT_d = nc.dram_tensor("WvT", [DC, DEO], BF16, kind="ExternalInput").ap()
    woT_d = nc.dram_tensor("WoT", [DEO, DE], BF16, kind="ExternalInput").ap()
    bq_d = nc.dram_tensor("bq", [DEO], F32, kind="ExternalInput").ap()
    bk_d = nc.dram_tensor("bk", [DEO], F32, kind="ExternalInput").ap()
    out_d = nc.dram_tensor("out", [S, DE], F32, kind="ExternalOutput").ap()

    with tile.TileContext(nc) as tc:
        with (
            tc.tile_pool(name="misc", bufs=1) as misc,
            tc.tile_pool(name="attn_keep", bufs=1) as attn_keep,
        ):
            ones_sb = misc.tile([128, 128], BF16)
            nc.any.memset(ones_sb, 1.0)
            bq_sb = misc.tile([128, NDI], F32)
            nc.sync.dma_start(bq_sb, bq_d.rearrange("(j p) -> p j", p=128))
            bk_sb = misc.tile([128, NDI], F32)
            nc.sync.dma_start(bk_sb, bk_d.rearrange("(j p) -> p j", p=128))

            attnT = attn_keep.tile([128, NDI, S], BF16)
            woT = attn_keep.tile([128, NDI, DE], BF16)

            with tc.tile_pool(name="q_keep", bufs=1) as q_keep:
                qT = q_keep.tile([128, NDI, S], BF16)
                kv_keep = tc.alloc_tile_pool(name="kv_keep", bufs=1)
                kT = kv_keep.tile([128, NDI, SKV], BF16)
                v_sb = kv_keep.tile([128, NT, DEO], BF16)

                w_in = tc.alloc_tile_pool(name="w_in", bufs=1)
                wqT = w_in.tile([128, NE, DEO], BF16)
                yT = w_in.tile([128, NCC, SKV], BF16)
                wkT = w_in.tile([128, NCC, DEO], BF16)
                wvT = w_in.tile([128, NCC, DEO], BF16)
                x_in = tc.alloc_tile_pool(name="x_in", bufs=1)
                xT = x_in.tile([128, NE, S], BF16)

                # DMA order = need order: (wq, x chunk 0) interleaved for
                # the ei-outer quick start, rest of x chunk-major, then k/v
                # weights, then wo — all on one queue so nothing contends
                # with the quick-start set.
                for i in range(NE):
                    nc.gpsimd.dma_start(wqT[:, i], wqT_d[ts(i, 128)])
                    nc.gpsimd.dma_start(
                        xT[:, i, 0:512], xT_d[ts(i, 128), 0:512]
                    )
                for c in range(1, NCH):
                    for i in range(NE):
                        nc.gpsimd.dma_start(
                            xT[:, i, ts(c, 512)], xT_d[ts(i, 128), ts(c, 512)]
                        )
                for i in range(NCC):
                    nc.gpsimd.dma_start(wkT[:, i], wkT_d[ts(i, 128)])
                    nc.gpsimd.dma_start(yT[:, i], yT_d[ts(i, 128)])
                for i in range(NCC):
                    nc.gpsimd.dma_start(wvT[:, i], wvT_d[ts(i, 128)])
                for i in range(NDI):
                    nc.gpsimd.dma_start(woT[:, i], woT_d[ts(i, 128)])

                # ---- stage Q ----
                with tc.tile_pool(name="ps_q", bufs=8, space="PSUM") as ps_q:
                    # chunk 0: ei-outer so the first matmul only needs
                    # wq[0] + x[0, :384] to have landed
                    ps0 = [
                        ps_q.tile([128, 512], F32, tag="psq", name=f"psq0_{i}")
                        for i in range(NDI)
                    ]
                    for ei in range(NE):
                        for di in range(NDI):
                            nc.tensor.matmul(
                                ps0[di],
                                wqT[:, ei, ts(di, 128)],
                                xT[:, ei, 0:512],
                                start=(ei == 0),
                                stop=(ei == NE - 1),
                            )
                    for di in range(NDI):
                        nc.scalar.activation(
                            qT[:, di, 0:512], ps0[di],
                            mybir.ActivationFunctionType.Identity,
                            bias=bq_sb[:, ds(di, 1)],
                        )
                    # chunks 1..5: di-outer (x already resident)
                    for sc in range(1, NCH):
                        for di in range(NDI):
                            ps = ps_q.tile([128, 512], F32, tag="psq")
                            for ei in range(NE):
                                nc.tensor.matmul(
                                    ps,
                                    wqT[:, ei, ts(di, 128)],
                                    xT[:, ei, ts(sc, 512)],
                                    start=(ei == 0),
                                    stop=(ei == NE - 1),
                                )
                            nc.scalar.activation(
                                qT[:, di, ts(sc, 512)], ps,
                                mybir.ActivationFunctionType.Identity,
                                bias=bq_sb[:, ds(di, 1)],
                            )
                x_in.release()

                # ---- stage K ----
                with tc.tile_pool(name="ps_k", bufs=8, space="PSUM") as ps_k:
                    for di in range(NDI):
                        for tc_i in range(3):
                            t0, tw = tc_i * 512, min(512, SKV - tc_i * 512)
                            ps = ps_k.tile([128, 512], F32, tag="psk")
                            for ci in range(NCC):
                                nc.tensor.matmul(
                                    ps[:, :tw],
                                    wkT[:, ci, ts(di, 128)],
                                    yT[:, ci, ds(t0, tw)],
                                    start=(ci == 0),
                                    stop=(ci == NCC - 1),
                                )
                            nc.scalar.activation(
                                kT[:, di, ds(t0, tw)], ps[:, :tw],
                                mybir.ActivationFunctionType.Identity,
                                bias=bk_sb[:, ds(di, 1)],
                            )

                # ---- stage V ----
                with tc.tile_pool(name="ps_v", bufs=8, space="PSUM") as ps_v:
                    nc.vector.memset(v_sb[:, NT_FULL], 0.0)
                    for ti in range(NT):
                        tw = _t_width(ti)
                        ps = ps_v.tile([128, 512], F32, tag="psv")
                        for ci in range(NCC):
                            nc.tensor.matmul(
                                ps[:tw],
                                yT[:, ci, ds(ti * 128, tw)],
                                wvT[:, ci],
                                start=(ci == 0),
                                stop=(ci == NCC - 1),
                            )
                        nc.vector.tensor_copy(v_sb[:tw, ti], ps[:tw])

                # ---- attention + streamed output projection ----
                with (
                    tc.tile_pool(name="awork", bufs=2) as awork,
                    tc.tile_pool(name="o_out", bufs=3) as o_out,
                    tc.tile_pool(name="ps_s", bufs=2, space="PSUM") as ps_s,
                    tc.tile_pool(name="ps_o", bufs=1, space="PSUM") as ps_o,
                    tc.tile_pool(name="ps_z", bufs=1, space="PSUM") as ps_z,
                    tc.tile_pool(name="ps_f", bufs=2, space="PSUM") as ps_f,
                ):
                    # persistent tail-exp tiles: partitions >= T_REM are
                    # memset to zero once and never rewritten
                    e8a = awork.tile([128, 512], BF16, tag="e8a", bufs=1)
                    e8b = awork.tile([128, 512], BF16, tag="e8b", bufs=1)
                    nc.vector.memset(e8a, 0.0)
                    nc.vector.memset(e8b, 0.0)
                    e8s = [e8a, e8b]

                    def scores_tile(h, sc, pi, expT, e8):
                        # one scores psum tile (2 t-tiles, or the 12-row
                        # tail) + its exp drain
                        ps = ps_s.tile([128, 2, 512], F32, tag="pss")
                        nj = 2 if pi < 4 else 1
                        for j in range(nj):
                            ti = pi * 2 + j
                            tw = _t_width(ti)
                            nc.tensor.matmul(
                                ps[:tw, j],
                                kT[:, h, ds(ti * 128, tw)],
                                qT[:, h, ts(sc, 512)],
                                start=True,
                                stop=True,
                            )
                        if nj == 2:
                            nc.scalar.activation(
                                expT[:, ts(pi, 2)], ps,
                                mybir.ActivationFunctionType.Exp,
                                scale=INV_SQRT_DH,
                            )
                        else:
                            nc.scalar.activation(
                                e8[:T_REM], ps[:T_REM, 0],
                                mybir.ActivationFunctionType.Exp,
                                scale=INV_SQRT_DH,
                            )

                    def pv_part(h, pso, expT, e8, lo, hi):
                        for ti in range(lo, hi):
                            nc.tensor.matmul(
                                pso,
                                v_sb[:, ti, ds(h * 128, 128)],
                                expT[:, ti] if ti < NT_FULL else e8,
                                start=(ti == 0),
                                stop=(ti == NT_FULL),
                            )

                    def emit_adds(expT, e8):
                        # bf16 tree folds all 9 tiles for one ones-matmul
                        sA = awork.tile([128, 512], BF16, tag="sA")
                        sB = awork.tile([128, 512], BF16, tag="sB")
                        sC = awork.tile([128, 512], BF16, tag="sC")
                        sD = awork.tile([128, 512], BF16, tag="sD")
                        nc.vector.tensor_tensor(sA, expT[:, 0], expT[:, 1], op=mybir.AluOpType.add)
                        nc.vector.tensor_tensor(sB, expT[:, 2], expT[:, 3], op=mybir.AluOpType.add)
                        nc.vector.tensor_tensor(sC, expT[:, 4], expT[:, 5], op=mybir.AluOpType.add)
                        nc.vector.tensor_tensor(sD, expT[:, 6], expT[:, 7], op=mybir.AluOpType.add)
                        nc.vector.tensor_tensor(sD, sD, e8, op=mybir.AluOpType.add)
                        nc.vector.tensor_tensor(sA, sA, sB, op=mybir.AluOpType.add)
                        nc.vector.tensor_tensor(sC, sC, sD, op=mybir.AluOpType.add)
                        nc.vector.tensor_tensor(sA, sA, sC, op=mybir.AluOpType.add)
                        return sA

                    def stage_norm(h, sc, zf, pso):
                        zrb = awork.tile([128, 512], F32, tag="zrb")
                        nc.vector.reciprocal_approx_fast(zrb, zf)
                        nc.vector.tensor_tensor(
                            attnT[:, h, ts(sc, 512)],
                            pso, zrb,
                            op=mybir.AluOpType.mult,
                        )

                    def op_group(si, g, out_sb):
                        # half of one partial-out s-tile (bias added on host)
                        ps = ps_f.tile([128, 512], F32, tag="psf")
                        for di in range(NDI):
                            nc.tensor.matmul(
                                ps,
                                attnT[:, di, ts(si, 128)],
                                woT[:, di, ts(g, 512)],
                                start=(di == 0),
                                stop=(di == NDI - 1),
                            )
                        nc.vector.tensor_copy(out_sb[:, ts(g, 512)], ps)
                        nc.sync.dma_start(
                            out_d[ts(si, 128), ts(g, 512)],
                            out_sb[:, ts(g, 512)],
                        )

                    def emit_outproj(si):
                        out_sb = o_out.tile([128, DE], F32, tag="osb")
                        op_group(si, 0, out_sb)
                        op_group(si, 1, out_sb)

                    # Interleaved schedule: within each iteration, PV and
                    # out-proj matmuls (whose inputs are >=1 iteration old)
                    # fill PE while ACT's exp chain catches up, so scores
                    # tile k+2 never waits on exp(k)'s psum-bank release.
                    iters = [(sc, h) for sc in range(NSC) for h in range(HC)]
                    op_queue = []
                    pipeA = None
                    pipeB = None
                    for it, (sc, h) in enumerate(iters):
                        op_si = op_queue.pop(0) if op_queue else None
                        expT = awork.tile(
                            [128, NT_FULL, 512], BF16, tag="expT",
                            name=f"expT_{it}",
                        )
                        e8 = e8s[it % 2]
                        scores_tile(h, sc, 0, expT, e8)
                        scores_tile(h, sc, 1, expT, e8)
                        if pipeB is not None:
                            stage_norm(*pipeB)
                            pipeB = None
                        if pipeA is not None:
                            sA_prev = emit_adds(pipeA[2], pipeA[3])
                        if op_si is not None:
                            out_sb = o_out.tile([128, DE], F32, tag="osb")
                            op_group(op_si, 0, out_sb)
                        if pipeA is not None:
                            pso = ps_o.tile([128, 512], F32, tag="pso")
                            pv_part(pipeA[0], pso, pipeA[2], pipeA[3], 0, 3)
                        scores_tile(h, sc, 2, expT, e8)
                        if pipeA is not None:
                            pv_part(pipeA[0], pso, pipeA[2], pipeA[3], 3, 6)
                        scores_tile(h, sc, 3, expT, e8)
                        if pipeA is not None:
                            pv_part(pipeA[0], pso, pipeA[2], pipeA[3], 6, NT)
                        scores_tile(h, sc, 4, expT, e8)
                        if op_si is not None:
                            op_group(op_si, 1, out_sb)
                        if pipeA is not None:
                            zf = ps_z.tile([128, 512], F32, tag="zf")
                            nc.tensor.matmul(zf, ones_sb, sA_prev, start=True, stop=True)
                            pipeB = (pipeA[0], pipeA[1], zf, pso)
                        pipeA = (h, sc, expT, e8)
                        # attnT[:, :, sc-1] completes (norm of its h=3) at
                        # the end of (sc, h=1); queue its 4 out tiles then
                        if h == 1 and sc >= 1:
                            op_queue.extend(range((sc - 1) * 4, sc * 4))

                    # drain: PV/Z of the last iteration, final norms, then
                    # the leftover out tiles
                    sA_prev = emit_adds(pipeA[2], pipeA[3])
                    pso = ps_o.tile([128, 512], F32, tag="pso")
                    pv_part(pipeA[0], pso, pipeA[2], pipeA[3], 0, NT)
                    zf = ps_z.tile([128, 512], F32, tag="zf")
                    nc.tensor.matmul(zf, ones_sb, sA_prev, start=True, stop=True)
                    stage_norm(*pipeB)
                    stage_norm(pipeA[0], pipeA[1], zf, pso)
                    for si in op_queue:
                        emit_outproj(si)
                    for si in range((NSC - 1) * 4, NSC * 4):
                        emit_outproj(si)
                w_in.release()
                kv_keep.release()

    nc.compile()
    return nc


_NC_CACHE = None


def _get_nc():
    global _NC_CACHE
    if _NC_CACHE is None:
        _NC_CACHE = build_bass()
    return _NC_CACHE


def make_in_maps(inputs):
    bf16 = ml_dtypes.bfloat16
    x = np.asarray(inputs["x"], np.float32)
    y = np.asarray(inputs["y"], np.float32)
    Wq = np.asarray(inputs["Wq"], np.float32)
    Wk = np.asarray(inputs["Wk"], np.float32)
    Wv = np.asarray(inputs["Wv"], np.float32)
    Wo = np.asarray(inputs["Wo"], np.float32)
    bq = np.asarray(inputs["bq"], np.float32)
    bk = np.asarray(inputs["bk"], np.float32)
    bv = np.asarray(inputs["bv"], np.float32)
    bo = np.asarray(inputs["bo"], np.float32)

    global _CST
    _CST = (Wo @ bv + bo).astype(np.float32)
    WqT = np.ascontiguousarray(Wq.T).astype(bf16)
    WkT = np.ascontiguousarray(Wk.T).astype(bf16)
    WvT = np.ascontiguousarray(Wv.T).astype(bf16)
    WoT = np.ascontiguousarray(Wo.T).astype(bf16)

    halves = []
    for hh in range(2):
        sl = slice(hh * DEO, (hh + 1) * DEO)
        halves.append(
            {
                "WqT": np.ascontiguousarray(WqT[:, sl]),
                "WkT": np.ascontiguousarray(WkT[:, sl]),
                "WvT": np.ascontiguousarray(WvT[:, sl]),
                "WoT": np.ascontiguousarray(WoT[sl, :]),
                "bq": np.ascontiguousarray(bq[sl]),
                "bk": np.ascontiguousarray(bk[sl]),
            }
        )
    xTs = [np.ascontiguousarray(x[b].T).astype(bf16) for b in range(B)]
    yTs = [np.ascontiguousarray(y[b].T).astype(bf16) for b in range(B)]

    in_maps = []
    for c in range(N_CORES):
        b, hh = c // 2, c % 2
        in_maps.append({"xT": xTs[b], "yT": yTs[b], **halves[hh]})
    return in_maps


_CST = None


def gather(results):
    out = np.empty((B, S, DE), np.float32)
    for c in range(0, N_CORES, 2):
        b = c // 2
        out[b] = results[c]["out"]
        out[b] += results[c + 1]["out"]
    out += _CST  # output-projection bias, folded on host
    return out


def kernel(**inputs) -> np.ndarray:
    nc = _get_nc()
    in_maps = make_in_maps(inputs)
    res = run_bass_kernel_spmd(nc, in_maps, core_ids=list(range(N_CORES)))
    return gather(res.results)


# revision 36
# speedup vs baseline: 1.0228x; 1.0228x over previous
"""Cross-attention Trainium2 kernel (8 NeuronCores, SPMD).

Problem: B=4, S=3072, SKV=1036, D_EMBED=1024, D_CROSS=768, H=8, d_head=128.

Sharding (per spec hint): data-parallel over batch x tensor-parallel over
heads. Core c -> (batch b = c//2, head half hh = c%2). Each core projects
only its 4 heads' Q/K/V dims (halves K/V projection work), runs attention
for its 4 heads over the full 3072 query rows, and applies the row-split
output projection (contraction over its own 512 embed dims) producing a
PARTIAL [3072, 1024] output. The all-reduce of the two partials per batch
happens in gather() on host (cores cannot talk to each other here).

Per-core device program (feature-on-partition, token-on-free):
  inputs arrive pre-cast to bf16 on host (halves HBM ingest, keeps DMAs on
  the hardware-descriptor path instead of gpsimd software casting).
  qT[d,s]  = WqT.T @ xT    (bf16; chunk 0 ei-outer so PE starts after
                            1/8th of Wq+x has landed)
  kT[d,t]  = WkT.T @ yT
  v[t,d]   = yT.T @ WvT
  attention, sc-outer / head-inner, 3-stage software pipeline:
    scores[t,s] = kT_h.T @ qT_h_slice       (psum)
    p = exp(scores / sqrt(128))             (ACT -> bf16 expT)
    Z row sums: bf16 DVE tree (7 adds) -> 2 ones-matmuls (psum zf)
    attnT_h[:, sc] = (sum_t v_h[t].T @ pT[t]) * 1/zf  (DVE reads psums)
  out[s,e] partial = attn[s, own] @ WoT[own] (+ cstB on the hh=0 core);
    emitted per finished s-chunk so the 12.6MB output streams during
    attention instead of draining at the end.
"""

import sys

sys.path.insert(0, "/opt/trn_rl_repo")

import math

import numpy as np
import ml_dtypes

import concourse.bass as bass
import concourse.mybir as mybir
import concourse.tile as tile
from concourse import bacc
from concourse.bass import ts, ds
from concourse.bass_utils import run_bass_kernel_spmd

N_CORES = 8
B, S, SKV = 4, 3072, 1036
DE, DC, H, DH = 1024, 768, 8, 128
HC = H // 2            # 4 heads per core
DEO = HC * DH          # 512 own embed dims
NT_FULL = SKV // 128   # 8 full t-tiles
T_REM = SKV - NT_FULL * 128  # 12
NT = NT_FULL + 1       # 9 t-tiles
NE = DE // 128         # 8 contraction chunks (embed)
NDI = DEO // 128       # 4 own-dim chunks
NCC = DC // 128        # 6 contraction chunks (cross)
NCH = S // 512         # 6 x-chunks of 512 (ap 512 hides LDWEIGHTS)
NSC = S // 512         # 6 s-chunks of 512
NS = S // 128          # 24 s-tiles
INV_SQRT_DH = 1.0 / math.sqrt(DH)

F32 = mybir.dt.float32
F32R = mybir.dt.float32r
BF16 = mybir.dt.bfloat16


def _t_width(ti):
    return 128 if ti < NT_FULL else T_REM


def build_bass():
    nc = bacc.Bacc("TRN2", target_bir_lowering=False, debug=False)

    xT_d = nc.dram_tensor("xT", [DE, S], BF16, kind="ExternalInput").ap()
    yT_d = nc.dram_tensor("yT", [DC, SKV], BF16, kind="ExternalInput").ap()
    wqT_d = nc.dram_tensor("WqT", [DE, DEO], BF16, kind="ExternalInput").ap()
    wkT_d = nc.dram_tensor("WkT", [DC, DEO], BF16, kind="ExternalInput").ap()
    wvT_d = nc.dram_tensor("WvT", [DC, DEO], BF16, kind="ExternalInput").ap()
    woT_d = nc.dram_tensor("WoT", [DEO, DE], BF16, kind="ExternalInput").ap()
    bq_d = nc.dram_tensor("bq", [DEO], F32, kind="ExternalInput").ap()
    bk_d = nc.dram_tensor("bk", [DEO], F32, kind="ExternalInput").ap()
    out_d = nc.dram_tensor("out", [S, DE], F32, kind="ExternalOutput").ap()

    with tile.TileContext(nc) as tc:
        with (
            tc.tile_pool(name="misc", bufs=1) as misc,
            tc.tile_pool(name="attn_keep", bufs=1) as attn_keep,
        ):
            ones_sb = misc.tile([128, 128], BF16)
            nc.any.memset(ones_sb, 1.0)
            bq_sb = misc.tile([128, NDI], F32)
            nc.sync.dma_start(bq_sb, bq_d.rearrange("(j p) -> p j", p=128))
            bk_sb = misc.tile([128, NDI], F32)
            nc.sync.dma_start(bk_sb, bk_d.rearrange("(j p) -> p j", p=128))

            attnT = attn_keep.tile([128, NDI, S], BF16)
            woT = attn_keep.tile([128, NDI, DE], BF16)

            with tc.tile_pool(name="q_keep", bufs=1) as q_keep:
                qT = q_keep.tile([128, NDI, S], BF16)
                kv_keep = tc.alloc_tile_pool(name="kv_keep", bufs=1)
                kT = kv_keep.tile([128, NDI, SKV], BF16)
                v_sb = kv_keep.tile([128, NT, DEO], BF16)

                w_in = tc.alloc_tile_pool(name="w_in", bufs=1)
                wqT = w_in.tile([128, NE, DEO], BF16)
                yT = w_in.tile([128, NCC, SKV], BF16)
                wkT = w_in.tile([128, NCC, DEO], BF16)
                wvT = w_in.tile([128, NCC, DEO], BF16)
                x_in = tc.alloc_tile_pool(name="x_in", bufs=1)
                xT = x_in.tile([128, NE, S], BF16)

                # DMA order = need order: (wq, x chunk 0) interleaved for
                # the ei-outer quick start, rest of x chunk-major, then k/v
                # weights, then wo — all on one queue so nothing contends
                # with the quick-start set.
                for i in range(NE):
                    nc.gpsimd.dma_start(wqT[:, i], wqT_d[ts(i, 128)])
                    nc.gpsimd.dma_start(
                        xT[:, i, 0:512], xT_d[ts(i, 128), 0:512]
                    )
                for c in range(1, NCH):
                    for i in range(NE):
                        nc.gpsimd.dma_start(
                            xT[:, i, ts(c, 512)], xT_d[ts(i, 128), ts(c, 512)]
                        )
                for i in range(NCC):
                    nc.gpsimd.dma_start(wkT[:, i], wkT_d[ts(i, 128)])
                    nc.gpsimd.dma_start(yT[:, i], yT_d[ts(i, 128)])
                for i in range(NCC):
                    nc.gpsimd.dma_start(wvT[:, i], wvT_d[ts(i, 128)])
                for i in range(NDI):
                    nc.gpsimd.dma_start(woT[:, i], woT_d[ts(i, 128)])

                # ---- stages Q, K, V share one psum pool: a single tag
                # rotation avoids pool-transition barriers between phases
                proj_pool = tc.alloc_tile_pool(
                    name="ps_proj", bufs=8, space="PSUM"
                )
                if True:
                    ps_q = ps_k = ps_v = proj_pool
                    # chunk 0: ei-outer so the first matmul only needs
                    # wq[0] + x[0, :512] to have landed
                    ps0 = [
                        ps_q.tile([128, 512], F32, tag="psq", name=f"psq0_{i}")
                        for i in range(NDI)
                    ]
                    for ei in range(NE):
                        for di in range(NDI):
                            nc.tensor.matmul(
                                ps0[di],
                                wqT[:, ei, ts(di, 128)],
                                xT[:, ei, 0:512],
                                start=(ei == 0),
                                stop=(ei == NE - 1),
                            )
                    for di in range(NDI):
                        nc.scalar.activation(
                            qT[:, di, 0:512], ps0[di],
                            mybir.ActivationFunctionType.Identity,
                            bias=bq_sb[:, ds(di, 1)],
                        )
                    # chunks 1..5: di-outer (x already resident)
                    for sc in range(1, NCH):
                        for di in range(NDI):
                            ps = ps_q.tile([128, 512], F32, tag="psq")
                            for ei in range(NE):
                                nc.tensor.matmul(
                                    ps,
                                    wqT[:, ei, ts(di, 128)],
                                    xT[:, ei, ts(sc, 512)],
                                    start=(ei == 0),
                                    stop=(ei == NE - 1),
                                )
                            nc.scalar.activation(
                                qT[:, di, ts(sc, 512)], ps,
                                mybir.ActivationFunctionType.Identity,
                                bias=bq_sb[:, ds(di, 1)],
                            )
                x_in.release()

                # ---- stage K ----
                if True:
                    for di in range(NDI):
                        for tc_i in range(3):
                            t0, tw = tc_i * 512, min(512, SKV - tc_i * 512)
                            ps = ps_k.tile([128, 512], F32, tag="psq")
                            for ci in range(NCC):
                                nc.tensor.matmul(
                                    ps[:, :tw],
                                    wkT[:, ci, ts(di, 128)],
                                    yT[:, ci, ds(t0, tw)],
                                    start=(ci == 0),
                                    stop=(ci == NCC - 1),
                                )
                            nc.scalar.activation(
                                kT[:, di, ds(t0, tw)], ps[:, :tw],
                                mybir.ActivationFunctionType.Identity,
                                bias=bk_sb[:, ds(di, 1)],
                            )

                # ---- stage V ----
                if True:
                    nc.vector.memset(v_sb[:, NT_FULL], 0.0)
                    for ti in range(NT):
                        tw = _t_width(ti)
                        ps = ps_v.tile([128, 512], F32, tag="psq")
                        for ci in range(NCC):
                            nc.tensor.matmul(
                                ps[:tw],
                                yT[:, ci, ds(ti * 128, tw)],
                                wvT[:, ci],
                                start=(ci == 0),
                                stop=(ci == NCC - 1),
                            )
                        nc.vector.tensor_copy(v_sb[:tw, ti], ps[:tw])
                proj_pool.release()

                # ---- attention + streamed output projection ----
                with (
                    tc.tile_pool(name="awork", bufs=2) as awork,
                    tc.tile_pool(name="o_out", bufs=3) as o_out,
                    tc.tile_pool(name="ps_s", bufs=2, space="PSUM") as ps_s,
                    tc.tile_pool(name="ps_o", bufs=1, space="PSUM") as ps_o,
                    tc.tile_pool(name="ps_z", bufs=1, space="PSUM") as ps_z,
                    tc.tile_pool(name="ps_f", bufs=2, space="PSUM") as ps_f,
                ):
                    # persistent tail-exp tiles: partitions >= T_REM are
                    # memset to zero once and never rewritten
                    e8a = awork.tile([128, 512], BF16, tag="e8a", bufs=1)
                    e8b = awork.tile([128, 512], BF16, tag="e8b", bufs=1)
                    nc.vector.memset(e8a, 0.0)
                    nc.vector.memset(e8b, 0.0)
                    e8s = [e8a, e8b]

                    def scores_tile(h, sc, pi, expT, e8):
                        # one scores psum tile (2 t-tiles, or the 12-row
                        # tail) + its exp drain
                        ps = ps_s.tile([128, 2, 512], F32, tag="pss")
                        nj = 2 if pi < 4 else 1
                        for j in range(nj):
                            ti = pi * 2 + j
                            tw = _t_width(ti)
                            nc.tensor.matmul(
                                ps[:tw, j],
                                kT[:, h, ds(ti * 128, tw)],
                                qT[:, h, ts(sc, 512)],
                                start=True,
                                stop=True,
                            )
                        if nj == 2:
                            nc.scalar.activation(
                                expT[:, ts(pi, 2)], ps,
                                mybir.ActivationFunctionType.Exp,
                                scale=INV_SQRT_DH,
                            )
                        else:
                            nc.scalar.activation(
                                e8[:T_REM], ps[:T_REM, 0],
                                mybir.ActivationFunctionType.Exp,
                                scale=INV_SQRT_DH,
                            )

                    def pv_part(h, pso, expT, e8, lo, hi):
                        for ti in range(lo, hi):
                            nc.tensor.matmul(
                                pso,
                                v_sb[:, ti, ds(h * 128, 128)],
                                expT[:, ti] if ti < NT_FULL else e8,
                                start=(ti == 0),
                                stop=(ti == NT_FULL),
                            )

                    def emit_adds(expT, e8):
                        # bf16 tree folds all 9 tiles for one ones-matmul
                        sA = awork.tile([128, 512], BF16, tag="sA")
                        sB = awork.tile([128, 512], BF16, tag="sB")
                        sC = awork.tile([128, 512], BF16, tag="sC")
                        sD = awork.tile([128, 512], BF16, tag="sD")
                        nc.vector.tensor_tensor(sA, expT[:, 0], expT[:, 1], op=mybir.AluOpType.add)
                        nc.vector.tensor_tensor(sB, expT[:, 2], expT[:, 3], op=mybir.AluOpType.add)
                        nc.vector.tensor_tensor(sC, expT[:, 4], expT[:, 5], op=mybir.AluOpType.add)
                        nc.vector.tensor_tensor(sD, expT[:, 6], expT[:, 7], op=mybir.AluOpType.add)
                        nc.vector.tensor_tensor(sD, sD, e8, op=mybir.AluOpType.add)
                        nc.vector.tensor_tensor(sA, sA, sB, op=mybir.AluOpType.add)
                        nc.vector.tensor_tensor(sC, sC, sD, op=mybir.AluOpType.add)
                        nc.vector.tensor_tensor(sA, sA, sC, op=mybir.AluOpType.add)
                        return sA

                    def stage_norm(h, sc, zf, pso):
                        zrb = awork.tile([128, 512], F32, tag="zrb")
                        nc.vector.reciprocal_approx_fast(zrb, zf)
                        nc.vector.tensor_tensor(
                            attnT[:, h, ts(sc, 512)],
                            pso, zrb,
                            op=mybir.AluOpType.mult,
                        )

                    def op_group(si, g, out_sb):
                        # half of one partial-out s-tile (bias added on host)
                        ps = ps_f.tile([128, 512], F32, tag="psf")
                        for di in range(NDI):
                            nc.tensor.matmul(
                                ps,
                                attnT[:, di, ts(si, 128)],
                                woT[:, di, ts(g, 512)],
                                start=(di == 0),
                                stop=(di == NDI - 1),
                            )
                        nc.vector.tensor_copy(out_sb[:, ts(g, 512)], ps)
                        nc.sync.dma_start(
                            out_d[ts(si, 128), ts(g, 512)],
                            out_sb[:, ts(g, 512)],
                        )

                    def emit_outproj(si):
                        out_sb = o_out.tile([128, DE], F32, tag="osb")
                        op_group(si, 0, out_sb)
                        op_group(si, 1, out_sb)

                    # Interleaved schedule: within each iteration, PV and
                    # out-proj matmuls (whose inputs are >=1 iteration old)
                    # fill PE while ACT's exp chain catches up, so scores
                    # tile k+2 never waits on exp(k)'s psum-bank release.
                    iters = [(sc, h) for sc in range(NSC) for h in range(HC)]
                    op_queue = []
                    pipeA = None
                    pipeB = None
                    for it, (sc, h) in enumerate(iters):
                        op_si = op_queue.pop(0) if op_queue else None
                        expT = awork.tile(
                            [128, NT_FULL, 512], BF16, tag="expT",
                            name=f"expT_{it}",
                        )
                        e8 = e8s[it % 2]
                        scores_tile(h, sc, 0, expT, e8)
                        scores_tile(h, sc, 1, expT, e8)
                        if pipeB is not None:
                            stage_norm(*pipeB)
                            pipeB = None
                        if pipeA is not None:
                            sA_prev = emit_adds(pipeA[2], pipeA[3])
                        if op_si is not None:
                            out_sb = o_out.tile([128, DE], F32, tag="osb")
                            op_group(op_si, 0, out_sb)
                        if pipeA is not None:
                            pso = ps_o.tile([128, 512], F32, tag="pso")
                            pv_part(pipeA[0], pso, pipeA[2], pipeA[3], 0, 3)
                        scores_tile(h, sc, 2, expT, e8)
                        if pipeA is not None:
                            pv_part(pipeA[0], pso, pipeA[2], pipeA[3], 3, 6)
                        scores_tile(h, sc, 3, expT, e8)
                        if pipeA is not None:
                            pv_part(pipeA[0], pso, pipeA[2], pipeA[3], 6, NT)
                        scores_tile(h, sc, 4, expT, e8)
                        if op_si is not None:
                            op_group(op_si, 1, out_sb)
                        if pipeA is not None:
                            zf = ps_z.tile([128, 512], F32, tag="zf")
                            nc.tensor.matmul(zf, ones_sb, sA_prev, start=True, stop=True)
                            pipeB = (pipeA[0], pipeA[1], zf, pso)
                        pipeA = (h, sc, expT, e8)
                        # attnT[:, :, sc-1] completes (norm of its h=3) at
                        # the end of (sc, h=1); queue its 4 out tiles then
                        if h == 1 and sc >= 1:
                            op_queue.extend(range((sc - 1) * 4, sc * 4))

                    # drain: PV/Z of the last iteration, final norms, then
                    # the leftover out tiles
                    sA_prev = emit_adds(pipeA[2], pipeA[3])
                    pso = ps_o.tile([128, 512], F32, tag="pso")
                    pv_part(pipeA[0], pso, pipeA[2], pipeA[3], 0, NT)
                    zf = ps_z.tile([128, 512], F32, tag="zf")
                    nc.tensor.matmul(zf, ones_sb, sA_prev, start=True, stop=True)
                    stage_norm(*pipeB)
                    stage_norm(pipeA[0], pipeA[1], zf, pso)
                    for si in op_queue:
                        emit_outproj(si)
                    for si in range((NSC - 1) * 4, NSC * 4):
                        emit_outproj(si)
                w_in.release()
                kv_keep.release()

    nc.compile()
    return nc


_NC_CACHE = None


def _get_nc():
    global _NC_CACHE
    if _NC_CACHE is None:
        _NC_CACHE = build_bass()
    return _NC_CACHE


def make_in_maps(inputs):
    bf16 = ml_dtypes.bfloat16
    x = np.asarray(inputs["x"], np.float32)
    y = np.asarray(inputs["y"], np.float32)
    Wq = np.asarray(inputs["Wq"], np.float32)
    Wk = np.asarray(inputs["Wk"], np.float32)
    Wv = np.asarray(inputs["Wv"], np.float32)
    Wo = np.asarray(inputs["Wo"], np.float32)
    bq = np.asarray(inputs["bq"], np.float32)
    bk = np.asarray(inputs["bk"], np.float32)
    bv = np.asarray(inputs["bv"], np.float32)
    bo = np.asarray(inputs["bo"], np.float32)

    global _CST
    _CST = (Wo @ bv + bo).astype(np.float32)
    WqT = np.ascontiguousarray(Wq.T).astype(bf16)
    WkT = np.ascontiguousarray(Wk.T).astype(bf16)
    WvT = np.ascontiguousarray(Wv.T).astype(bf16)
    WoT = np.ascontiguousarray(Wo.T).astype(bf16)

    halves = []
    for hh in range(2):
        sl = slice(hh * DEO, (hh + 1) * DEO)
        halves.append(
            {
                "WqT": np.ascontiguousarray(WqT[:, sl]),
                "WkT": np.ascontiguousarray(WkT[:, sl]),
                "WvT": np.ascontiguousarray(WvT[:, sl]),
                "WoT": np.ascontiguousarray(WoT[sl, :]),
                "bq": np.ascontiguousarray(bq[sl]),
                "bk": np.ascontiguousarray(bk[sl]),
            }
        )
    xTs = [np.ascontiguousarray(x[b].T).astype(bf16) for b in range(B)]
    yTs = [np.ascontiguousarray(y[b].T).astype(bf16) for b in range(B)]

    in_maps = []
    for c in range(N_CORES):
        b, hh = c // 2, c % 2
        in_maps.append({"xT": xTs[b], "yT": yTs[b], **halves[hh]})
    return in_maps


_CST = None


def gather(results):
    out = np.empty((B, S, DE), np.float32)
    for c in range(0, N_CORES, 2):
        b = c // 2
        out[b] = results[c]["out"]
        out[b] += results[c + 1]["out"]
    out += _CST  # output-projection bias, folded on host
    return out


def kernel(**inputs) -> np.ndarray:
    nc = _get_nc()
    in_maps = make_in_maps(inputs)
    res = run_bass_kernel_spmd(nc, in_maps, core_ids=list(range(N_CORES)))
    return gather(res.results)
